# revision 1
# baseline (speedup 1.0000x reference)
"""Low_Rank_linear Trainium2 kernel.

Math (reference):
    hidden = (x[..., col_idx] * wnorm) @ B.T            # [tok, 512]
    y[..., row_idx]      = hidden @ A.T + x[..., col_comp_idx] @ sparse1.T
    y[..., row_comp_idx] = x @ sparse2.T

Reformulation used here (all index handling folded into host-built weights):
    u = x @ W1.T        W1 = [Bs; G; sparse2]  (1024 x 4096)
        Bs[:, col_idx]        = B * wnorm      (rank rows scattered to full width)
        G[i, col_comp_idx[i]] = 1              (one-hot gather of comp columns)
    y = u @ W2.T        W2 (4096 x 1024), rows interleaved on host:
        W2[row_idx[j]]      = [A[j] | sparse1[j] | 0]
        W2[row_comp_idx[i]] = [0    | 0          | e_i]
    so y comes out of the second matmul already in natural column order.

Sharding: data-parallel over the 8192 tokens -> 1024 tokens per core, weights
replicated. No collectives. Matmuls run in bf16 with fp32 PSUM accumulation.
"""

import numpy as np
import ml_dtypes

import concourse.bacc as bacc
import concourse.tile as tile
import concourse.mybir as mybir
from concourse.bass_utils import run_bass_kernel_spmd

N_CORES = 8
TOK = 8192            # 4 * 2048 tokens total
TPC = TOK // N_CORES  # 1024 tokens per core
N = 4096              # model width (in == out)
RANK = 512
NCOMP = 256           # complement set size (both col and row)
KU = RANK + NCOMP + NCOMP  # 1024 = width of intermediate u
BLK = 512             # token block (matmul moving N)
TT = 128              # token tile (partition dim)

_BF16 = mybir.dt.bfloat16
_F32 = mybir.dt.float32


def _build_nc():
    nc = bacc.Bacc(None)
    x_d = nc.dram_tensor("x", [TPC, N], _F32, kind="ExternalInput")
    w1_d = nc.dram_tensor("w1t", [N, KU], _BF16, kind="ExternalInput")
    w2_d = nc.dram_tensor("w2t", [KU, N], _BF16, kind="ExternalInput")
    y_d = nc.dram_tensor("y", [TPC, N], _F32, kind="ExternalOutput")

    n_blk = TPC // BLK          # 2 token blocks per core
    tpb = BLK // TT             # 4 token tiles per block
    k1 = N // 128               # 32 k-tiles for matmul A
    m1 = KU // 128              # 8 u-feature tiles
    k2 = KU // 128              # 8 k-tiles for matmul B
    n2 = N // BLK               # 8 output-feature chunks

    with tile.TileContext(nc) as tc:
        with (
            tc.tile_pool(name="w1", bufs=1) as w1_pool,
            tc.tile_pool(name="w2", bufs=2) as w2_pool,
            tc.tile_pool(name="xb", bufs=2) as xb_pool,
            tc.tile_pool(name="xt", bufs=2) as xt_pool,
            tc.tile_pool(name="u3", bufs=2) as u3_pool,
            tc.tile_pool(name="yo", bufs=4) as yo_pool,
            tc.tile_pool(name="psA", bufs=2, space="PSUM") as psA,
            tc.tile_pool(name="psB", bufs=2, space="PSUM") as psB,
        ):
            # resident W1.T in SBUF: [128, 32 k-tiles, 1024]
            w1_sb = w1_pool.tile([128, k1, KU], _BF16)
            nc.sync.dma_start(
                w1_sb[:], w1_d.rearrange("(kt p) m -> p kt m", p=128)
            )

            for blk in range(n_blk):
                t0 = blk * BLK
                # load + cast x to bf16 (token-major), then DMA-transpose to
                # feature-major xt [128 feat, k-tile, 512 tok]
                xt_sb = xt_pool.tile([128, k1, BLK], _BF16)
                for tt in range(tpb):
                    xb = xb_pool.tile([128, N], _BF16)
                    nc.gpsimd.dma_start(
                        xb[:], x_d[t0 + tt * TT : t0 + (tt + 1) * TT, :]
                    )
                    nc.sync.dma_start_transpose(
                        xt_sb[:, :, tt * TT : (tt + 1) * TT], xb[:]
                    )

                # MM-A: u.T [ufeat, tok] = W1 @ x.T ; cast to bf16
                u3_sb = u3_pool.tile([128, k2, BLK], _BF16)
                for m in range(m1):
                    ps = psA.tile([128, BLK], _F32)
                    for kt in range(k1):
                        nc.tensor.matmul(
                            ps[:],
                            w1_sb[:, kt, m * 128 : (m + 1) * 128],
                            xt_sb[:, kt, :],
                            start=(kt == 0),
                            stop=(kt == k1 - 1),
                        )
                    nc.vector.tensor_copy(out=u3_sb[:, m, :], in_=ps[:])

                # MM-B: y [tok, outfeat] = u @ W2.T, n-chunk at a time
                for n in range(n2):
                    w2_sb = w2_pool.tile([128, k2, BLK], _BF16)
                    nc.sync.dma_start(
                        w2_sb[:],
                        w2_d.rearrange("(kt p) n -> p kt n", p=128)[
                            :, :, n * BLK : (n + 1) * BLK
                        ],
                    )
                    for mt in range(tpb):
                        ps = psB.tile([128, BLK], _F32)
                        for kt in range(k2):
                            nc.tensor.matmul(
                                ps[:],
                                u3_sb[:, kt, mt * TT : (mt + 1) * TT],
                                w2_sb[:, kt, :],
                                start=(kt == 0),
                                stop=(kt == k2 - 1),
                            )
                        yo = yo_pool.tile([128, BLK], _F32)
                        nc.vector.tensor_copy(out=yo[:], in_=ps[:])
                        nc.sync.dma_start(
                            y_d[
                                t0 + mt * TT : t0 + (mt + 1) * TT,
                                n * BLK : (n + 1) * BLK,
                            ],
                            yo[:],
                        )
    nc.finalize()
    return nc


_NC_CACHE = {}


def get_nc():
    if "nc" not in _NC_CACHE:
        _NC_CACHE["nc"] = _build_nc()
    return _NC_CACHE["nc"]


def _prep_weights(A, B, sparse_weights1, sparse_weights2, weights_norms_rowwise,
                  col_idx, col_comp_idx, row_idx, row_comp_idx):
    bf16 = ml_dtypes.bfloat16
    # W1 = [Bs; G; sparse2]  (1024, 4096)
    w1 = np.zeros((KU, N), dtype=np.float32)
    w1[:RANK, col_idx] = B * weights_norms_rowwise[None, :]
    w1[RANK + np.arange(NCOMP), col_comp_idx] = 1.0
    w1[RANK + NCOMP :, :] = sparse_weights2
    # W2 (4096, 1024) with interleaved rows; build transposed directly
    w2t = np.zeros((KU, N), dtype=np.float32)
    w2t[:RANK, row_idx] = A.T
    w2t[RANK : RANK + NCOMP, row_idx] = sparse_weights1.T
    w2t[RANK + NCOMP + np.arange(NCOMP), row_comp_idx] = 1.0
    w1t = np.ascontiguousarray(w1.T).astype(bf16)       # [4096, 1024]
    w2t = np.ascontiguousarray(w2t).astype(bf16)        # [1024, 4096]
    return w1t, w2t


def kernel(x, A, B, sparse_weights1, sparse_weights2, weights_norms_rowwise,
           col_idx, col_comp_idx, row_idx, row_comp_idx):
    x = np.asarray(x, dtype=np.float32)
    w1t, w2t = _prep_weights(
        np.asarray(A, np.float32), np.asarray(B, np.float32),
        np.asarray(sparse_weights1, np.float32),
        np.asarray(sparse_weights2, np.float32),
        np.asarray(weights_norms_rowwise, np.float32),
        np.asarray(col_idx), np.asarray(col_comp_idx),
        np.asarray(row_idx), np.asarray(row_comp_idx),
    )
    nc = get_nc()
    xs = np.ascontiguousarray(x.reshape(TOK, N))
    in_maps = [
        {"x": xs[c * TPC : (c + 1) * TPC], "w1t": w1t, "w2t": w2t}
        for c in range(N_CORES)
    ]
    res = run_bass_kernel_spmd(nc, in_maps, core_ids=list(range(N_CORES)))
    globals()["_LAST_RESULTS"] = res
    y = np.concatenate([res.results[c]["y"] for c in range(N_CORES)], axis=0)
    return np.ascontiguousarray(y.reshape(x.shape).astype(np.float32))

